# revision 23
# baseline (speedup 1.0000x reference)
"""Causal self-attention head (softmax over the QUERY axis) on 8 trn2 cores.

Reference math (softmax axis=-2, i.e. per key-column):
    q = x @ Wq; k = x @ Wk; v = x @ Wv            # [B,T,64]
    s[b,q,k] = (q . k) * 64**-0.5, masked to q >= k
    w[:, k]  = softmax over q of s[:, k]           # column softmax
    out[b,q,:] = sum_k w[q,k] v[k,:]

The normalizer folds into a per-key scaling of v:
    out[q] = sum_{k<=q} exp(s[q,k]) * (r[k] * v[k]),  r[k] = 1/sum_{q>=k} exp(s[q,k])

Sharding: 8 cores = 4 batches x 2 "parities". Core (b, p) owns the key-column
blocks kb = 2i+p (i=0..15, 128 columns each) of batch b and produces a partial
output over all q; the host adds the two parity partials per batch. Parity-1
cores receive x^T shifted left by 128 columns (zero-padded tail) so the SPMD
program is identical on all cores; a per-core "tailmask" input kills the pad.

Design notes (the first working kernel was PE-column-bound at ~120us; this
version restructures for full PE-array utilization and engine balance):
- Projections use packed 128-wide stationaries so the full PE array works:
  [Wq|Wq] gives q^T twice (partitions 0:64 / 64:128 - the duplicate feeds the
  concurrent B-block score matmuls), [Wk|Wv] / [Wv|Wk] give each chunk's two
  key blocks k^T+v^T in one 128-col stream each, with block A's k^T landing at
  partitions 0:64 and block B's at 64:128. All weights ship as one packed
  dram tensor (single DMA ahead of the x chunks on the sync ring).
- Score matmuls for the two key blocks run CONCURRENTLY as row-tiled PE pairs
  (block A in array rows 0:63, block B in rows 64:127, delta-start ~4ns),
  halving score streaming time. Both blocks stream q over [512j, T); block B
  gets a 384-wide triangular mask (atriB @ bnegB count-matmul).
- Blocks A and B keep separate contiguous [128, 1024] psum score tiles
  (2-tile groups) so each block's exp is ONE contiguous ACTIVATE; per-key
  sums alternate between ACT's fused accumulator (+READ_ACCUMULATOR) and a
  DVE in-place tensor_scalar with fused accum_out, balancing both engines.
- Output stripes run as col-tiled concurrent pairs (stripe 2c at PSUM
  partitions 0:64, 2c+1 at 64:128 of one bank). Stripe contributions from
  pairs j>=1 are emitted as PE "filler" interleaved with the last pairs' exp
  tiles so the PE never starves and the HAM clock-gate stays at 2.4 GHz; the
  pair-0 contributions + flushes are the only serial tail, with stripes 0-3
  accumulating in the freed score banks in parallel with stripe 4-7 flushes.
- All 8 x-chunk DMAs are issued up front (descending) on the sync queue; a
  dependency-free PE warm-up burst opens the HAM clock-gate during the load,
  and a dummy Exp preloads the ACT table set during the ramp.
"""

import sys

import numpy as np

for _p in ("/opt/trn_rl_repo",):
    if _p not in sys.path:
        sys.path.insert(0, _p)

import concourse.bass as bass
import concourse.mybir as mybir
from concourse import bacc
from concourse.bass_utils import run_bass_kernel_spmd
from concourse.tile import TileContext

B, T, CE, CH = 4, 4096, 1024, 64
P = 128
NB = 16          # key blocks per core (128 cols each)
NCHUNK = 8       # 512-col chunks covering T
SCALE = CH ** -0.5
NEG = -1e30
M0 = NEG / P     # per-unit mask magnitude for the triangular-count mask
ET = 512         # per-block scores/exp tile width (1 psum bank)

F32 = mybir.dt.float32
BF16 = mybir.dt.bfloat16

N_CORES = 8

LAST_RESULTS = None


def _build_program():
    nc = bacc.Bacc("TRN2", target_bir_lowering=False, debug=False)

    xT = nc.declare_dram_parameter("xT", [CE, T], BF16, isOutput=False)
    wpk = nc.declare_dram_parameter("wpk", [CE, 3 * P], BF16, isOutput=False)
    tailmask = nc.declare_dram_parameter("tailmask", [P, P], BF16, isOutput=False)
    outT = nc.declare_dram_parameter("outT", [CH, T], F32, isOutput=True)

    with TileContext(nc) as tc:
        with (
            tc.tile_pool(name="consts", bufs=1) as consts,
            tc.tile_pool(name="qkv", bufs=1) as qkv,
            tc.tile_pool(name="w2pool", bufs=1) as w2pool,
            tc.tile_pool(name="xp", bufs=8) as xp,
            tc.tile_pool(name="projp", bufs=1, space="PSUM") as projp,
            tc.tile_pool(name="pop", bufs=2, space="PSUM") as pop,
            tc.tile_pool(name="scp", bufs=1, space="PSUM") as scp,
        ):
            # ---- DMA'd constants ----
            wpk_sb = consts.tile([P, CE // P, 3 * P], BF16, tag="wpk")
            nc.sync.dma_start(wpk_sb[:], wpk.rearrange("(o p) f -> p o f", p=P))
            wqq_sb = wpk_sb.rearrange("p o (w f) -> p o w f", w=3)[:, :, 0]
            wkv_sb = wpk_sb.rearrange("p o (w f) -> p o w f", w=3)[:, :, 1]
            wvk_sb = wpk_sb.rearrange("p o (w f) -> p o w f", w=3)[:, :, 2]
            tmask = consts.tile([P, P], BF16, tag="tmask")
            nc.sync.dma_start(tmask[:], tailmask[:])

            # all x chunk DMAs up front (descending), on the sync queue
            xtiles = [None] * NCHUNK
            for j in reversed(range(NCHUNK)):
                xtiles[j] = xp.tile(
                    [P, CE // P, 512], BF16, tag="xtile", name=f"x{j}"
                )
                nc.sync.dma_start(
                    xtiles[j][:],
                    xT[:, 512 * j:512 * (j + 1)].rearrange("(o p) f -> p o f", p=P),
                )

            # ---- gpsimd-built mask constants ----
            # atri[ch, p] = 1[ch < p];   bneg[ch, c] = M0 * 1[c <= ch]
            #   => (atri^T @ bneg)[p, c] = M0 * max(0, p - c)   (A-block mask)
            # atriB[ch, p] = 1[ch <= p]; bnegB[ch, c] = M0 * 1[c <= ch + 255]
            #   => (atriB^T @ bnegB)[p, c] < 0 iff c < 256 + p  (B-block mask)
            ones = consts.tile([P, P], BF16, tag="ones")
            nc.gpsimd.memset(ones[:], 1.0)
            atri = consts.tile([P, P], BF16, tag="atri")
            nc.gpsimd.memset(atri[:], 1.0)
            nc.gpsimd.affine_select(
                out=atri[:], in_=atri[:],
                compare_op=mybir.AluOpType.is_ge, fill=0.0,
                base=-1, pattern=[[1, P]], channel_multiplier=-1,
            )
            atriB = consts.tile([P, P], BF16, tag="atriB")
            nc.gpsimd.memset(atriB[:], 1.0)
            nc.gpsimd.affine_select(
                out=atriB[:], in_=atriB[:],
                compare_op=mybir.AluOpType.is_ge, fill=0.0,
                base=0, pattern=[[1, P]], channel_multiplier=-1,
            )
            bneg = consts.tile([P, 2 * P], BF16, tag="bneg")
            nc.gpsimd.memset(bneg[:], M0)
            nc.gpsimd.affine_select(
                out=bneg[:], in_=bneg[:],
                compare_op=mybir.AluOpType.is_ge, fill=0.0,
                base=0, pattern=[[-1, 2 * P]], channel_multiplier=1,
            )
            bnegB = consts.tile([P, 3 * P], BF16, tag="bnegB")
            nc.gpsimd.memset(bnegB[:], M0)
            nc.gpsimd.affine_select(
                out=bnegB[:], in_=bnegB[:],
                compare_op=mybir.AluOpType.is_ge, fill=0.0,
                base=255, pattern=[[-1, 3 * P]], channel_multiplier=1,
            )

            # ---- persistent activations ----
            # qTd: q^T duplicated across partition halves (B scores read 64:128)
            qTd = qkv.tile([P, T], BF16, tag="qTd")
            # kv[:, j, 0:128]: [0:64]=kT blk 2j,   [64:128]=vT blk 2j
            # kv[:, j, 128:256]: [0:64]=vT blk 2j+1, [64:128]=kT blk 2j+1
            kv = qkv.tile([P, NCHUNK, 2 * P], BF16, tag="kv")
            vnat = qkv.tile([P, NB, CH], BF16, tag="vnat")
            stats = qkv.tile([P, NCHUNK, 4, 2], F32, tag="stats")
            ssum = qkv.tile([P, NB], F32, tag="ssum")
            rr = qkv.tile([P, NB], F32, tag="rr")
            outsb = qkv.tile([P, 4 * 512], F32, tag="outsb")

            # w2 pair tiles: w2[j][:, 0, :] = block 2j, [:, 1, :] = block 2j+1,
            # both spanning q in [512j, T)
            w2 = [
                w2pool.tile([P, 2, T - 512 * j], BF16, tag=f"w2_{j}", name=f"w2_{j}")
                for j in range(NCHUNK)
            ]

            # PE warm-up spam keeps the HAM clock-gate opening while the
            # first DMAs land.
            # Calibrated warm-up burst: ~10-11us of dependency-free 256-col
            # matmuls keep the PE continuously busy (HAM clock-gate opens
            # after ~3.4us and STAYS open) until the first x chunk lands, so
            # the first projections run at 2.4 GHz instead of 1.2.
            for t in range(40):
                dscr = pop.tile([P, 512], F32, tag="po", name=f"warm{t}")
                nc.tensor.matmul(
                    dscr[0:P, 0:256], qTd[0:P, 0:P], qTd[0:P, 0:256],
                    start=True, stop=True,
                )
            dscr = pop.tile([P, 512], F32, tag="po", name="abs_tm")
            nc.tensor.matmul(
                dscr[0:1, 0:1], tmask[0:CH, 0:1], tmask[0:CH, 0:1],
                start=True, stop=True,
            )

            # preload the exp table set (~2.7us) during the ramp
            nc.scalar.activation(
                stats[0:1, 7, 3, 0:1], ones[0:1, 0:1],
                mybir.ActivationFunctionType.Exp, scale=1.0,
            )

            # filler queue: thunks emitting one output-stripe matmul each,
            # consumed between the last pairs' score tiles to keep PE fed
            fillers = []

            deferred_sums = []
            pending_fin = []

            def drain_fillers(k):
                while k > 0 and fillers:
                    fillers.pop(0)()
                    k -= 1

            def emit_pair(j, fill=0):
                # Blocks A (rows 0:63) and B (rows 64:127) get separate
                # contiguous 1024-wide psum tiles, processed in 2-tile groups.
                # Per-block ACTIVATEs then carry a correct fused accumulator;
                # sums alternate between ACT accum and DVE cache-reduce so
                # neither engine owns all of them.
                L = T - 512 * j
                nt = L // ET
                ngroups = (nt + 1) // 2
                # block B's first 256 cols are always fully masked: provide
                # the zeros via gpsimd and skip them in the exp activate
                nc.gpsimd.memset(w2[j][:, 1, 0:256], 0.0)
                for gi in range(ngroups):
                    t0 = 2 * gi
                    gw = min(2, nt - t0)
                    wA = ET * gw
                    scA = scp.tile([P, 2 * ET], F32, tag="scA",
                                   name=f"scA{j}_{gi}")
                    scB = scp.tile([P, 2 * ET], F32, tag="scB",
                                   name=f"scB{j}_{gi}")
                    pend = []  # (bankkey, kwargs)
                    for u in range(gw):
                        qs = 512 * j + ET * (t0 + u)
                        pend.append((("A", u), dict(
                            out=scA[:, ET * u:ET * (u + 1)],
                            lhsT=kv[0:CH, j, 0:P],
                            rhs=qTd[0:CH, qs:qs + ET], start=True)))
                        pend.append((("B", u), dict(
                            out=scB[:, ET * u:ET * (u + 1)],
                            lhsT=kv[CH:P, j, P:2 * P],
                            rhs=qTd[CH:P, qs:qs + ET], start=True)))
                    if t0 == 0:
                        pend.append((("A", 0), dict(out=scA[:, 0:256],
                                                    lhsT=atri[:],
                                                    rhs=bneg[:], start=False)))
                        pend.append((("B", 0), dict(out=scB[:, 0:3 * P],
                                                    lhsT=atriB[:],
                                                    rhs=bnegB[:],
                                                    start=False)))
                    if t0 + gw == nt:
                        pend.append((("A", gw - 1), dict(
                            out=scA[:, wA - P:wA], lhsT=ones[:],
                            rhs=tmask[:], start=False)))
                        pend.append((("B", gw - 1), dict(
                            out=scB[:, wA - P:wA], lhsT=ones[:],
                            rhs=tmask[:], start=False)))
                    last_per_bank = {}
                    for idx, (bank, kw) in enumerate(pend):
                        last_per_bank[bank] = idx
                    lasts = set(last_per_bank.values())
                    for idx, (bank, kw) in enumerate(pend):
                        nc.tensor.matmul(
                            kw["out"], kw["lhsT"], kw["rhs"],
                            start=kw["start"], stop=(idx in lasts),
                            skip_group_check=True,
                        )
                    lo = ET * t0
                    use_act = ((j + gi) % 2 == 1)
                    for a, sct in ((0, scA), (1, scB)):
                        skip = 256 if (a == 1 and gi == 0) else 0
                        nc.scalar.activation(
                            w2[j][:, a, lo + skip:lo + wA],
                            sct[:, skip:wA],
                            mybir.ActivationFunctionType.Exp, scale=SCALE,
                            accum_out=(stats[:, j, gi, a:a + 1]
                                       if use_act else None),
                        )
                        if not use_act:
                            if gi == ngroups - 1:
                                # deferred with the finalize, so the next
                                # chunk's casts lead the DVE queue
                                deferred_sums.append(
                                    (j, a, lo, wA, gi))
                            else:
                                nc.vector.tensor_scalar(
                                    out=w2[j][:, a, lo:lo + wA],
                                    in0=w2[j][:, a, lo:lo + wA],
                                    scalar1=1.0, scalar2=None,
                                    op0=mybir.AluOpType.mult,
                                    op1=mybir.AluOpType.add,
                                    accum_out=stats[:, j, gi, a:a + 1],
                                )
                    if gi == 0:
                        # previous pair's deferred finalize (sums/reciprocal/
                        # v-scale) lands here - behind this chunk's casts and
                        # this group's sums in the DVE FIFO, so it can't block
                        # the queue head, but still before any filler matmul
                        # that reads the scaled vnat
                        while pending_fin:
                            pending_fin.pop(0)()
                    drain_fillers(fill)

                def finalize():
                    for (jj, a, lo, wA, gi) in deferred_sums[:]:
                        if jj != j:
                            continue
                        deferred_sums.remove((jj, a, lo, wA, gi))
                        nc.vector.tensor_scalar(
                            out=w2[j][:, a, lo:lo + wA],
                            in0=w2[j][:, a, lo:lo + wA],
                            scalar1=1.0, scalar2=None,
                            op0=mybir.AluOpType.mult,
                            op1=mybir.AluOpType.add,
                            accum_out=stats[:, j, gi, a:a + 1],
                        )
                    for a in (0, 1):
                        i = 2 * j + a
                        nc.vector.reduce_sum(
                            ssum[:, i:i + 1], stats[:, j, 0:ngroups, a],
                            axis=mybir.AxisListType.X,
                        )
                        nc.vector.reciprocal(rr[:, i:i + 1], ssum[:, i:i + 1])
                        nc.vector.tensor_scalar_mul(
                            vnat[:, i, :], vnat[:, i, :], rr[:, i:i + 1]
                        )
                pending_fin.append(finalize)

            # ======== merged pipeline: chunks descending, scores inline ========
            def process_chunk(j, fill=0):
                xtile = xtiles[j]
                proj = projp.tile([P, 768], F32, tag="proj", name=f"proj{j}")
                # absorber: park this chunk's DMA wait on a throwaway MM
                nc.tensor.matmul(
                    proj[0:1, 0:1], xtile[:, 0, 0:1], xtile[:, 0, 0:1],
                    start=True, stop=True,
                )
                # q projection, duplicated across partition halves: [Wq|Wq]
                for s in range(CE // P):
                    nc.tensor.matmul(
                        proj[:, 0:512], wqq_sb[:, s, :], xtile[:, s, :],
                        start=(s == 0), stop=(s == CE // P - 1),
                        skip_group_check=True,
                    )
                # k/v for the chunk's two key blocks (chunk-local cols 0:128
                # and 256:384): [Wk|Wv] for block A, [Wv|Wk] for block B.
                # The kv matmuls are emitted BEFORE the q cast so the cast's
                # read of proj[0:512] doesn't serialize them (conservative
                # overlap check on the shared proj tile).
                for s in range(CE // P):
                    nc.tensor.matmul(
                        proj[:, 512:640], wkv_sb[:, s, :], xtile[:, s, 0:P],
                        start=(s == 0), stop=(s == CE // P - 1),
                        skip_group_check=True,
                    )
                for s in range(CE // P):
                    nc.tensor.matmul(
                        proj[:, 640:768], wvk_sb[:, s, :], xtile[:, s, 2 * P:3 * P],
                        start=False, stop=(s == CE // P - 1),
                        skip_group_check=True,
                    )
                nc.vector.tensor_copy(qTd[:, 512 * j:512 * (j + 1)], proj[:, 0:512])
                nc.vector.tensor_copy(kv[:, j, :], proj[:, 512:768])
                # v -> natural layout via DMA xbar transpose
                teng = nc.sync
                teng.dma_start_transpose(vnat[:, 2 * j, :], kv[CH:P, j, 0:P])
                teng.dma_start_transpose(vnat[:, 2 * j + 1, :], kv[0:CH, j, P:2 * P])
                emit_pair(j, fill=fill)

            # ===== output stripes (definitions; emitted interleaved) =====
            # stripe s covers q [512s, 512s+512); pair pc packs stripe 2pc at
            # psum partitions 0:64 and stripe 2pc+1 at 64:128 of one bank.
            po_tiles = {}
            po_started = {}

            def po_mm(pc, jj, a, s, base, is_last):
                def thunk():
                    if pc not in po_tiles:
                        if pc >= 2:
                            po_tiles[pc] = pop.tile(
                                [P, 512], F32, tag="po", name=f"po{pc}"
                            )
                        else:
                            po_tiles[pc] = scp.tile(
                                [P, 2 * ET], F32,
                                tag=("scA" if pc == 1 else "scB"),
                                name=f"po{pc}",
                            )
                        po_started[pc] = {0: False, 64: False}
                    po = po_tiles[pc]
                    off = 512 * (s - jj)
                    nc.tensor.matmul(
                        po[base:base + CH, 0:512],
                        vnat[:, 2 * jj + a, :],
                        w2[jj][:, a, off:off + 512],
                        start=not po_started[pc][base], stop=is_last,
                        skip_group_check=True,
                    )
                    po_started[pc][base] = True
                return thunk

            def po_flush(pc):
                po = po_tiles[pc]
                nc.vector.tensor_copy(
                    outsb[:, 512 * pc:512 * (pc + 1)], po[:, 0:512]
                )
                nc.sync.dma_start(
                    outT[:, 1024 * pc:1024 * pc + 512],
                    outsb[0:CH, 512 * pc:512 * (pc + 1)],
                )
                nc.sync.dma_start(
                    outT[:, 1024 * pc + 512:1024 * (pc + 1)],
                    outsb[CH:P, 512 * pc:512 * (pc + 1)],
                )

            def po_batch(pc, jj_range):
                out = []
                for jj in jj_range:
                    for a in (0, 1):
                        for s, base in ((2 * pc, 0), (2 * pc + 1, 64)):
                            if jj > s:
                                continue
                            is_last = (jj == 0) and (a == 1) and (
                                True  # jj descending ends at 0
                            )
                            out.append(po_mm(pc, jj, a, s, base, is_last))
                return out

            # pipeline: chunks descending; during the last two pairs feed the
            # PE with po3/po2 contributions from already-finished pairs
            for j in reversed(range(3, NCHUNK)):
                process_chunk(j)
            fillers.extend(po_batch(3, range(7, 2, -1)))  # jj = 7..3
            process_chunk(2, fill=2)
            fillers.extend(po_batch(3, [2]))
            fillers.extend(po_batch(2, range(5, 1, -1)))  # jj = 5..2
            process_chunk(1, fill=3)
            fillers.extend(po_batch(3, [1]))
            fillers.extend(po_batch(2, [1]))
            process_chunk(0, fill=6)
            while pending_fin:
                pending_fin.pop(0)()
            drain_fillers(len(fillers))
            # pair-0 contributions + flushes
            for thunk in po_batch(3, [0]):
                thunk()
            po_flush(3)
            for thunk in po_batch(2, [0]):
                thunk()
            po_flush(2)
            for pc in (1, 0):
                for thunk in po_batch(pc, range(2 * pc + 1, -1, -1)):
                    thunk()
                po_flush(pc)

    return nc


_PROGRAM = None


def _get_program():
    global _PROGRAM
    if _PROGRAM is None:
        nc = _build_program()
        nc.finalize()
        _PROGRAM = nc
    return _PROGRAM


def kernel(x, Wk, Wq, Wv, trace=False, trace_cores=None):
    global LAST_RESULTS
    x = np.asarray(x)
    Wk = np.asarray(Wk)
    Wq = np.asarray(Wq)
    Wv = np.asarray(Wv)

    import ml_dtypes

    bf = ml_dtypes.bfloat16
    wpk = np.ascontiguousarray(
        np.concatenate([Wq, Wq, Wk, Wv, Wv, Wk], axis=1).astype(bf))

    zeros_mask = np.zeros((P, P), bf)
    neg_mask = np.full((P, P), NEG / P, bf)

    in_maps = []
    for c in range(N_CORES):
        b, parity = c // 2, c % 2
        xTb = np.ascontiguousarray(x[b].T).astype(bf)  # [CE, T]
        if parity:
            xTb = np.concatenate([xTb[:, P:], np.zeros((CE, P), bf)], axis=1)
        in_maps.append(
            {
                "xT": np.ascontiguousarray(xTb),
                "wpk": wpk,
                "tailmask": neg_mask if parity else zeros_mask,
            }
        )

    nc = _get_program()
    res = run_bass_kernel_spmd(
        nc,
        in_maps,
        list(range(N_CORES)),
        trace=trace,
        **({"trace_cores": trace_cores} if trace_cores is not None else {}),
    )
    LAST_RESULTS = res

    out = np.zeros((B, T, CH), np.float32)
    for c in range(N_CORES):
        b, parity = c // 2, c % 2
        oT = np.asarray(res.results[c]["outT"], np.float32)  # [CH, T]
        if parity:
            out[b, P:, :] += oT[:, : T - P].T
        else:
            out[b] += oT.T
    return out


# revision 25
# speedup vs baseline: 1.0195x; 1.0195x over previous
"""Causal self-attention head (softmax over the QUERY axis) on 8 trn2 cores.

Reference math (softmax axis=-2, i.e. per key-column):
    q = x @ Wq; k = x @ Wk; v = x @ Wv            # [B,T,64]
    s[b,q,k] = (q . k) * 64**-0.5, masked to q >= k
    w[:, k]  = softmax over q of s[:, k]           # column softmax
    out[b,q,:] = sum_k w[q,k] v[k,:]

The normalizer folds into a per-key scaling of v:
    out[q] = sum_{k<=q} exp(s[q,k]) * (r[k] * v[k]),  r[k] = 1/sum_{q>=k} exp(s[q,k])

Sharding: 8 cores = 4 batches x 2 "parities". Core (b, p) owns the key-column
blocks kb = 2i+p (i=0..15, 128 columns each) of batch b and produces a partial
output over all q; the host adds the two parity partials per batch. Parity-1
cores receive x^T shifted left by 128 columns (zero-padded tail) so the SPMD
program is identical on all cores; a per-core "tailmask" input kills the pad.

Design notes (the first working kernel was PE-column-bound at ~120us; this
version restructures for full PE-array utilization and engine balance):
- Projections use packed 128-wide stationaries so the full PE array works:
  [Wq|Wq] gives q^T twice (partitions 0:64 / 64:128 - the duplicate feeds the
  concurrent B-block score matmuls), [Wk|Wv] / [Wv|Wk] give each chunk's two
  key blocks k^T+v^T in one 128-col stream each, with block A's k^T landing at
  partitions 0:64 and block B's at 64:128. All weights ship as one packed
  dram tensor (single DMA ahead of the x chunks on the sync ring).
- Score matmuls for the two key blocks run CONCURRENTLY as row-tiled PE pairs
  (block A in array rows 0:63, block B in rows 64:127, delta-start ~4ns),
  halving score streaming time. Both blocks stream q over [512j, T); block B
  gets a 384-wide triangular mask (atriB @ bnegB count-matmul).
- Blocks A and B keep separate contiguous [128, 1024] psum score tiles
  (2-tile groups) so each block's exp is ONE contiguous ACTIVATE; per-key
  sums alternate between ACT's fused accumulator (+READ_ACCUMULATOR) and a
  DVE in-place tensor_scalar with fused accum_out, balancing both engines.
- Output stripes run as col-tiled concurrent pairs (stripe 2c at PSUM
  partitions 0:64, 2c+1 at 64:128 of one bank). Stripe contributions from
  pairs j>=1 are emitted as PE "filler" interleaved with the last pairs' exp
  tiles so the PE never starves and the HAM clock-gate stays at 2.4 GHz; the
  pair-0 contributions + flushes are the only serial tail, with stripes 0-3
  accumulating in the freed score banks in parallel with stripe 4-7 flushes.
- All 8 x-chunk DMAs are issued up front (descending) on the sync queue; a
  dependency-free PE warm-up burst opens the HAM clock-gate during the load,
  and a dummy Exp preloads the ACT table set during the ramp.
"""

import sys

import numpy as np

for _p in ("/opt/trn_rl_repo",):
    if _p not in sys.path:
        sys.path.insert(0, _p)

import concourse.bass as bass
import concourse.mybir as mybir
from concourse import bacc
from concourse.bass_utils import run_bass_kernel_spmd
from concourse.tile import TileContext

B, T, CE, CH = 4, 4096, 1024, 64
P = 128
NB = 16          # key blocks per core (128 cols each)
NCHUNK = 8       # 512-col chunks covering T
SCALE = CH ** -0.5
NEG = -1e30
M0 = NEG / P     # per-unit mask magnitude for the triangular-count mask
ET = 512         # per-block scores/exp tile width (1 psum bank)

F32 = mybir.dt.float32
BF16 = mybir.dt.bfloat16

N_CORES = 8

LAST_RESULTS = None


def _build_program():
    nc = bacc.Bacc("TRN2", target_bir_lowering=False, debug=False)

    xT = nc.declare_dram_parameter("xT", [CE, T], BF16, isOutput=False)
    wpk = nc.declare_dram_parameter("wpk", [CE, 3 * P], BF16, isOutput=False)
    tailmask = nc.declare_dram_parameter("tailmask", [P, P], BF16, isOutput=False)
    outT = nc.declare_dram_parameter("outT", [CH, T], F32, isOutput=True)

    with TileContext(nc) as tc:
        with (
            tc.tile_pool(name="consts", bufs=1) as consts,
            tc.tile_pool(name="qkv", bufs=1) as qkv,
            tc.tile_pool(name="w2pool", bufs=1) as w2pool,
            tc.tile_pool(name="xp", bufs=8) as xp,
            tc.tile_pool(name="projp", bufs=1, space="PSUM") as projp,
            tc.tile_pool(name="pop", bufs=2, space="PSUM") as pop,
            tc.tile_pool(name="scp", bufs=1, space="PSUM") as scp,
        ):
            # ---- first x chunk leads the DMA ring (weights are only
            # needed once it has landed), then constants, then the rest ----
            xtiles = [None] * NCHUNK
            xtiles[NCHUNK - 1] = xp.tile(
                [P, CE // P, 512], BF16, tag="xtile", name=f"x{NCHUNK - 1}"
            )
            nc.sync.dma_start(
                xtiles[NCHUNK - 1][:],
                xT[:, 512 * (NCHUNK - 1):].rearrange("(o p) f -> p o f", p=P),
            )
            wpk_sb = consts.tile([P, CE // P, 3 * P], BF16, tag="wpk")
            nc.sync.dma_start(wpk_sb[:], wpk.rearrange("(o p) f -> p o f", p=P))
            wqq_sb = wpk_sb.rearrange("p o (w f) -> p o w f", w=3)[:, :, 0]
            wkv_sb = wpk_sb.rearrange("p o (w f) -> p o w f", w=3)[:, :, 1]
            wvk_sb = wpk_sb.rearrange("p o (w f) -> p o w f", w=3)[:, :, 2]
            tmask = consts.tile([P, P], BF16, tag="tmask")
            nc.sync.dma_start(tmask[:], tailmask[:])

            # remaining x chunk DMAs (descending), on the sync queue
            for j in reversed(range(NCHUNK - 1)):
                xtiles[j] = xp.tile(
                    [P, CE // P, 512], BF16, tag="xtile", name=f"x{j}"
                )
                nc.sync.dma_start(
                    xtiles[j][:],
                    xT[:, 512 * j:512 * (j + 1)].rearrange("(o p) f -> p o f", p=P),
                )

            # ---- gpsimd-built mask constants ----
            # atri[ch, p] = 1[ch < p];   bneg[ch, c] = M0 * 1[c <= ch]
            #   => (atri^T @ bneg)[p, c] = M0 * max(0, p - c)   (A-block mask)
            # atriB[ch, p] = 1[ch <= p]; bnegB[ch, c] = M0 * 1[c <= ch + 255]
            #   => (atriB^T @ bnegB)[p, c] < 0 iff c < 256 + p  (B-block mask)
            ones = consts.tile([P, P], BF16, tag="ones")
            nc.gpsimd.memset(ones[:], 1.0)
            atri = consts.tile([P, P], BF16, tag="atri")
            nc.gpsimd.memset(atri[:], 1.0)
            nc.gpsimd.affine_select(
                out=atri[:], in_=atri[:],
                compare_op=mybir.AluOpType.is_ge, fill=0.0,
                base=-1, pattern=[[1, P]], channel_multiplier=-1,
            )
            atriB = consts.tile([P, P], BF16, tag="atriB")
            nc.gpsimd.memset(atriB[:], 1.0)
            nc.gpsimd.affine_select(
                out=atriB[:], in_=atriB[:],
                compare_op=mybir.AluOpType.is_ge, fill=0.0,
                base=0, pattern=[[1, P]], channel_multiplier=-1,
            )
            bneg = consts.tile([P, 2 * P], BF16, tag="bneg")
            nc.gpsimd.memset(bneg[:], M0)
            nc.gpsimd.affine_select(
                out=bneg[:], in_=bneg[:],
                compare_op=mybir.AluOpType.is_ge, fill=0.0,
                base=0, pattern=[[-1, 2 * P]], channel_multiplier=1,
            )
            bnegB = consts.tile([P, 3 * P], BF16, tag="bnegB")
            nc.gpsimd.memset(bnegB[:], M0)
            nc.gpsimd.affine_select(
                out=bnegB[:], in_=bnegB[:],
                compare_op=mybir.AluOpType.is_ge, fill=0.0,
                base=255, pattern=[[-1, 3 * P]], channel_multiplier=1,
            )

            # ---- persistent activations ----
            # qTd: q^T duplicated across partition halves (B scores read 64:128)
            qTd = qkv.tile([P, T], BF16, tag="qTd")
            # kv[:, j, 0:128]: [0:64]=kT blk 2j,   [64:128]=vT blk 2j
            # kv[:, j, 128:256]: [0:64]=vT blk 2j+1, [64:128]=kT blk 2j+1
            kv = qkv.tile([P, NCHUNK, 2 * P], BF16, tag="kv")
            vnat = qkv.tile([P, NB, CH], BF16, tag="vnat")
            stats = qkv.tile([P, NCHUNK, 4, 2], F32, tag="stats")
            ssum = qkv.tile([P, NB], F32, tag="ssum")
            rr = qkv.tile([P, NB], F32, tag="rr")
            outsb = qkv.tile([P, 4 * 512], F32, tag="outsb")

            # w2 pair tiles: w2[j][:, 0, :] = block 2j, [:, 1, :] = block 2j+1,
            # both spanning q in [512j, T)
            w2 = [
                w2pool.tile([P, 2, T - 512 * j], BF16, tag=f"w2_{j}", name=f"w2_{j}")
                for j in range(NCHUNK)
            ]

            # PE warm-up spam keeps the HAM clock-gate opening while the
            # first DMAs land.
            # Calibrated warm-up burst: ~10-11us of dependency-free 256-col
            # matmuls keep the PE continuously busy (HAM clock-gate opens
            # after ~3.4us and STAYS open) until the first x chunk lands, so
            # the first projections run at 2.4 GHz instead of 1.2.
            for t in range(40):
                dscr = pop.tile([P, 512], F32, tag="po", name=f"warm{t}")
                nc.tensor.matmul(
                    dscr[0:P, 0:256], qTd[0:P, 0:P], qTd[0:P, 0:256],
                    start=True, stop=True,
                )
            dscr = pop.tile([P, 512], F32, tag="po", name="abs_tm")
            nc.tensor.matmul(
                dscr[0:1, 0:1], tmask[0:CH, 0:1], tmask[0:CH, 0:1],
                start=True, stop=True,
            )

            # preload the exp table set (~2.7us) during the ramp
            nc.scalar.activation(
                stats[0:1, 7, 3, 0:1], ones[0:1, 0:1],
                mybir.ActivationFunctionType.Exp, scale=1.0,
            )

            # filler queue: thunks emitting one output-stripe matmul each,
            # consumed between the last pairs' score tiles to keep PE fed
            fillers = []

            deferred_sums = []
            pending_fin = []

            def drain_fillers(k):
                while k > 0 and fillers:
                    fillers.pop(0)()
                    k -= 1

            def emit_pair(j, fill=0):
                # Blocks A (rows 0:63) and B (rows 64:127) get separate
                # contiguous 1024-wide psum tiles, processed in 2-tile groups.
                # Per-block ACTIVATEs then carry a correct fused accumulator;
                # sums alternate between ACT accum and DVE cache-reduce so
                # neither engine owns all of them.
                L = T - 512 * j
                nt = L // ET
                ngroups = (nt + 1) // 2
                # block B's first 256 cols are always fully masked: provide
                # the zeros via gpsimd and skip them in the exp activate
                nc.gpsimd.memset(w2[j][:, 1, 0:256], 0.0)
                for gi in range(ngroups):
                    t0 = 2 * gi
                    gw = min(2, nt - t0)
                    wA = ET * gw
                    scA = scp.tile([P, 2 * ET], F32, tag="scA",
                                   name=f"scA{j}_{gi}")
                    scB = scp.tile([P, 2 * ET], F32, tag="scB",
                                   name=f"scB{j}_{gi}")
                    pend = []  # (bankkey, kwargs)
                    for u in range(gw):
                        qs = 512 * j + ET * (t0 + u)
                        pend.append((("A", u), dict(
                            out=scA[:, ET * u:ET * (u + 1)],
                            lhsT=kv[0:CH, j, 0:P],
                            rhs=qTd[0:CH, qs:qs + ET], start=True)))
                        pend.append((("B", u), dict(
                            out=scB[:, ET * u:ET * (u + 1)],
                            lhsT=kv[CH:P, j, P:2 * P],
                            rhs=qTd[CH:P, qs:qs + ET], start=True)))
                    if t0 == 0:
                        pend.append((("A", 0), dict(out=scA[:, 0:256],
                                                    lhsT=atri[:],
                                                    rhs=bneg[:], start=False)))
                        pend.append((("B", 0), dict(out=scB[:, 0:3 * P],
                                                    lhsT=atriB[:],
                                                    rhs=bnegB[:],
                                                    start=False)))
                    if t0 + gw == nt:
                        pend.append((("A", gw - 1), dict(
                            out=scA[:, wA - P:wA], lhsT=ones[:],
                            rhs=tmask[:], start=False)))
                        pend.append((("B", gw - 1), dict(
                            out=scB[:, wA - P:wA], lhsT=ones[:],
                            rhs=tmask[:], start=False)))
                    last_per_bank = {}
                    for idx, (bank, kw) in enumerate(pend):
                        last_per_bank[bank] = idx
                    lasts = set(last_per_bank.values())
                    for idx, (bank, kw) in enumerate(pend):
                        nc.tensor.matmul(
                            kw["out"], kw["lhsT"], kw["rhs"],
                            start=kw["start"], stop=(idx in lasts),
                            skip_group_check=True,
                        )
                    lo = ET * t0
                    use_act = ((j + gi) % 2 == 1)
                    for a, sct in ((0, scA), (1, scB)):
                        skip = 256 if (a == 1 and gi == 0) else 0
                        nc.scalar.activation(
                            w2[j][:, a, lo + skip:lo + wA],
                            sct[:, skip:wA],
                            mybir.ActivationFunctionType.Exp, scale=SCALE,
                            accum_out=(stats[:, j, gi, a:a + 1]
                                       if use_act else None),
                        )
                        if not use_act:
                            if gi == ngroups - 1:
                                # deferred with the finalize, so the next
                                # chunk's casts lead the DVE queue
                                deferred_sums.append(
                                    (j, a, lo, wA, gi))
                            else:
                                nc.vector.tensor_scalar(
                                    out=w2[j][:, a, lo:lo + wA],
                                    in0=w2[j][:, a, lo:lo + wA],
                                    scalar1=1.0, scalar2=None,
                                    op0=mybir.AluOpType.mult,
                                    op1=mybir.AluOpType.add,
                                    accum_out=stats[:, j, gi, a:a + 1],
                                )
                    drain_fillers(fill)

                def finalize():
                    for (jj, a, lo, wA, gi) in deferred_sums[:]:
                        if jj != j:
                            continue
                        deferred_sums.remove((jj, a, lo, wA, gi))
                        nc.vector.tensor_scalar(
                            out=w2[j][:, a, lo:lo + wA],
                            in0=w2[j][:, a, lo:lo + wA],
                            scalar1=1.0, scalar2=None,
                            op0=mybir.AluOpType.mult,
                            op1=mybir.AluOpType.add,
                            accum_out=stats[:, j, gi, a:a + 1],
                        )
                    for a in (0, 1):
                        i = 2 * j + a
                        nc.vector.reduce_sum(
                            ssum[:, i:i + 1], stats[:, j, 0:ngroups, a],
                            axis=mybir.AxisListType.X,
                        )
                        nc.vector.reciprocal(rr[:, i:i + 1], ssum[:, i:i + 1])
                        nc.vector.tensor_scalar_mul(
                            vnat[:, i, :], vnat[:, i, :], rr[:, i:i + 1]
                        )
                pending_fin.append(finalize)

            # ======== merged pipeline: chunks descending, scores inline ========
            def process_chunk(j, fill=0):
                xtile = xtiles[j]
                proj = projp.tile([P, 768], F32, tag="proj", name=f"proj{j}")
                # absorber: park this chunk's DMA wait on a throwaway MM
                nc.tensor.matmul(
                    proj[0:1, 0:1], xtile[:, 0, 0:1], xtile[:, 0, 0:1],
                    start=True, stop=True,
                )
                # q projection, duplicated across partition halves: [Wq|Wq]
                for s in range(CE // P):
                    nc.tensor.matmul(
                        proj[:, 0:512], wqq_sb[:, s, :], xtile[:, s, :],
                        start=(s == 0), stop=(s == CE // P - 1),
                        skip_group_check=True,
                    )
                # k/v for the chunk's two key blocks (chunk-local cols 0:128
                # and 256:384): [Wk|Wv] for block A, [Wv|Wk] for block B.
                # The kv matmuls are emitted BEFORE the q cast so the cast's
                # read of proj[0:512] doesn't serialize them (conservative
                # overlap check on the shared proj tile).
                for s in range(CE // P):
                    nc.tensor.matmul(
                        proj[:, 512:640], wkv_sb[:, s, :], xtile[:, s, 0:P],
                        start=(s == 0), stop=(s == CE // P - 1),
                        skip_group_check=True,
                    )
                for s in range(CE // P):
                    nc.tensor.matmul(
                        proj[:, 640:768], wvk_sb[:, s, :], xtile[:, s, 2 * P:3 * P],
                        start=False, stop=(s == CE // P - 1),
                        skip_group_check=True,
                    )
                nc.vector.tensor_copy(qTd[:, 512 * j:512 * (j + 1)], proj[:, 0:512])
                nc.vector.tensor_copy(kv[:, j, :], proj[:, 512:768])
                # previous pair's deferred finalize (sums/reciprocal/v-scale)
                # runs AFTER this chunk's casts so the casts lead the DVE
                # queue and the next scores/exp aren't stalled behind it
                while pending_fin:
                    pending_fin.pop(0)()
                # v -> natural layout via DMA xbar transpose
                teng = nc.sync
                teng.dma_start_transpose(vnat[:, 2 * j, :], kv[CH:P, j, 0:P])
                teng.dma_start_transpose(vnat[:, 2 * j + 1, :], kv[0:CH, j, P:2 * P])
                emit_pair(j, fill=fill)

            # ===== output stripes (definitions; emitted interleaved) =====
            # stripe s covers q [512s, 512s+512); pair pc packs stripe 2pc at
            # psum partitions 0:64 and stripe 2pc+1 at 64:128 of one bank.
            po_tiles = {}
            po_started = {}

            def po_mm(pc, jj, a, s, base, is_last):
                def thunk():
                    if pc not in po_tiles:
                        if pc >= 2:
                            po_tiles[pc] = pop.tile(
                                [P, 512], F32, tag="po", name=f"po{pc}"
                            )
                        else:
                            po_tiles[pc] = scp.tile(
                                [P, 2 * ET], F32,
                                tag=("scA" if pc == 1 else "scB"),
                                name=f"po{pc}",
                            )
                        po_started[pc] = {0: False, 64: False}
                    po = po_tiles[pc]
                    off = 512 * (s - jj)
                    nc.tensor.matmul(
                        po[base:base + CH, 0:512],
                        vnat[:, 2 * jj + a, :],
                        w2[jj][:, a, off:off + 512],
                        start=not po_started[pc][base], stop=is_last,
                        skip_group_check=True,
                    )
                    po_started[pc][base] = True
                return thunk

            def po_flush(pc):
                po = po_tiles[pc]
                nc.vector.tensor_copy(
                    outsb[:, 512 * pc:512 * (pc + 1)], po[:, 0:512]
                )
                nc.sync.dma_start(
                    outT[:, 1024 * pc:1024 * pc + 512],
                    outsb[0:CH, 512 * pc:512 * (pc + 1)],
                )
                nc.sync.dma_start(
                    outT[:, 1024 * pc + 512:1024 * (pc + 1)],
                    outsb[CH:P, 512 * pc:512 * (pc + 1)],
                )

            def po_batch(pc, jj_range):
                out = []
                for jj in jj_range:
                    for a in (0, 1):
                        for s, base in ((2 * pc, 0), (2 * pc + 1, 64)):
                            if jj > s:
                                continue
                            is_last = (jj == 0) and (a == 1) and (
                                True  # jj descending ends at 0
                            )
                            out.append(po_mm(pc, jj, a, s, base, is_last))
                return out

            # pipeline: chunks descending; during the last two pairs feed the
            # PE with po3/po2 contributions from already-finished pairs
            for j in reversed(range(3, NCHUNK)):
                process_chunk(j)
            fillers.extend(po_batch(3, range(7, 2, -1)))  # jj = 7..3
            process_chunk(2, fill=2)
            fillers.extend(po_batch(3, [2]))
            fillers.extend(po_batch(2, range(5, 1, -1)))  # jj = 5..2
            process_chunk(1, fill=3)
            fillers.extend(po_batch(3, [1]))
            fillers.extend(po_batch(2, [1]))
            process_chunk(0, fill=6)
            while pending_fin:
                pending_fin.pop(0)()
            drain_fillers(len(fillers))
            # pair-0 contributions + flushes
            for thunk in po_batch(3, [0]):
                thunk()
            po_flush(3)
            for thunk in po_batch(2, [0]):
                thunk()
            po_flush(2)
            for pc in (1, 0):
                for thunk in po_batch(pc, range(2 * pc + 1, -1, -1)):
                    thunk()
                po_flush(pc)

    return nc


_PROGRAM = None


def _get_program():
    global _PROGRAM
    if _PROGRAM is None:
        nc = _build_program()
        nc.finalize()
        _PROGRAM = nc
    return _PROGRAM


def kernel(x, Wk, Wq, Wv, trace=False, trace_cores=None):
    global LAST_RESULTS
    x = np.asarray(x)
    Wk = np.asarray(Wk)
    Wq = np.asarray(Wq)
    Wv = np.asarray(Wv)

    import ml_dtypes

    bf = ml_dtypes.bfloat16
    wpk = np.ascontiguousarray(
        np.concatenate([Wq, Wq, Wk, Wv, Wv, Wk], axis=1).astype(bf))

    zeros_mask = np.zeros((P, P), bf)
    neg_mask = np.full((P, P), NEG / P, bf)

    in_maps = []
    for c in range(N_CORES):
        b, parity = c // 2, c % 2
        xTb = np.ascontiguousarray(x[b].T).astype(bf)  # [CE, T]
        if parity:
            xTb = np.concatenate([xTb[:, P:], np.zeros((CE, P), bf)], axis=1)
        in_maps.append(
            {
                "xT": np.ascontiguousarray(xTb),
                "wpk": wpk,
                "tailmask": neg_mask if parity else zeros_mask,
            }
        )

    nc = _get_program()
    res = run_bass_kernel_spmd(
        nc,
        in_maps,
        list(range(N_CORES)),
        trace=trace,
        **({"trace_cores": trace_cores} if trace_cores is not None else {}),
    )
    LAST_RESULTS = res

    out = np.zeros((B, T, CH), np.float32)
    for c in range(N_CORES):
        b, parity = c // 2, c % 2
        oT = np.asarray(res.results[c]["outT"], np.float32)  # [CH, T]
        if parity:
            out[b, P:, :] += oT[:, : T - P].T
        else:
            out[b] += oT.T
    return out


# revision 26
# speedup vs baseline: 1.0538x; 1.0336x over previous
"""Causal self-attention head (softmax over the QUERY axis) on 8 trn2 cores.

Reference math (softmax axis=-2, i.e. per key-column):
    q = x @ Wq; k = x @ Wk; v = x @ Wv            # [B,T,64]
    s[b,q,k] = (q . k) * 64**-0.5, masked to q >= k
    w[:, k]  = softmax over q of s[:, k]           # column softmax
    out[b,q,:] = sum_k w[q,k] v[k,:]

The normalizer folds into a per-key scaling of v:
    out[q] = sum_{k<=q} exp(s[q,k]) * (r[k] * v[k]),  r[k] = 1/sum_{q>=k} exp(s[q,k])

Sharding: 8 cores = 4 batches x 2 "parities". Core (b, p) owns the key-column
blocks kb = 2i+p (i=0..15, 128 columns each) of batch b and produces a partial
output over all q; the host adds the two parity partials per batch. Parity-1
cores receive x^T shifted left by 128 columns (zero-padded tail) so the SPMD
program is identical on all cores; a per-core "tailmask" input kills the pad.

Design notes (the first working kernel was PE-column-bound at ~120us; this
version restructures for full PE-array utilization and engine balance):
- Projections use packed 128-wide stationaries so the full PE array works:
  [Wq|Wq] gives q^T twice (partitions 0:64 / 64:128 - the duplicate feeds the
  concurrent B-block score matmuls), [Wk|Wv] / [Wv|Wk] give each chunk's two
  key blocks k^T+v^T in one 128-col stream each, with block A's k^T landing at
  partitions 0:64 and block B's at 64:128. All weights ship as one packed
  dram tensor (single DMA ahead of the x chunks on the sync ring).
- Score matmuls for the two key blocks run CONCURRENTLY as row-tiled PE pairs
  (block A in array rows 0:63, block B in rows 64:127, delta-start ~4ns),
  halving score streaming time. Both blocks stream q over [512j, T); block B
  gets a 384-wide triangular mask (atriB @ bnegB count-matmul).
- Blocks A and B keep separate contiguous [128, 1024] psum score tiles
  (2-tile groups) so each block's exp is ONE contiguous ACTIVATE; per-key
  sums alternate between ACT's fused accumulator (+READ_ACCUMULATOR) and a
  DVE in-place tensor_scalar with fused accum_out, balancing both engines.
- Output stripes run as col-tiled concurrent pairs (stripe 2c at PSUM
  partitions 0:64, 2c+1 at 64:128 of one bank). Stripe contributions from
  pairs j>=1 are emitted as PE "filler" interleaved with the last pairs' exp
  tiles so the PE never starves and the HAM clock-gate stays at 2.4 GHz; the
  pair-0 contributions + flushes are the only serial tail, with stripes 0-3
  accumulating in the freed score banks in parallel with stripe 4-7 flushes.
- All 8 x-chunk DMAs are issued up front (descending) on the sync queue; a
  dependency-free PE warm-up burst opens the HAM clock-gate during the load,
  and a dummy Exp preloads the ACT table set during the ramp.
"""

import sys

import numpy as np

for _p in ("/opt/trn_rl_repo",):
    if _p not in sys.path:
        sys.path.insert(0, _p)

import concourse.bass as bass
import concourse.mybir as mybir
from concourse import bacc
from concourse.bass_utils import run_bass_kernel_spmd
from concourse.tile import TileContext

B, T, CE, CH = 4, 4096, 1024, 64
P = 128
NB = 16          # key blocks per core (128 cols each)
NCHUNK = 8       # 512-col chunks covering T
SCALE = CH ** -0.5
NEG = -1e30
M0 = NEG / P     # per-unit mask magnitude for the triangular-count mask
ET = 512         # per-block scores/exp tile width (1 psum bank)

F32 = mybir.dt.float32
BF16 = mybir.dt.bfloat16

N_CORES = 8

LAST_RESULTS = None


def _build_program():
    nc = bacc.Bacc("TRN2", target_bir_lowering=False, debug=False)

    xT = nc.declare_dram_parameter("xT", [CE, T], BF16, isOutput=False)
    wpk = nc.declare_dram_parameter("wpk", [CE, 3 * P], BF16, isOutput=False)
    tailmask = nc.declare_dram_parameter("tailmask", [P, P], BF16, isOutput=False)
    outT = nc.declare_dram_parameter("outT", [CH, T], F32, isOutput=True)

    with TileContext(nc) as tc:
        with (
            tc.tile_pool(name="consts", bufs=1) as consts,
            tc.tile_pool(name="qkv", bufs=1) as qkv,
            tc.tile_pool(name="w2pool", bufs=1) as w2pool,
            tc.tile_pool(name="xp", bufs=8) as xp,
            tc.tile_pool(name="projp", bufs=1, space="PSUM") as projp,
            tc.tile_pool(name="pop", bufs=2, space="PSUM") as pop,
            tc.tile_pool(name="scp", bufs=1, space="PSUM") as scp,
        ):
            # ---- DMA'd constants ----
            wpk_sb = consts.tile([P, CE // P, 3 * P], BF16, tag="wpk")
            nc.sync.dma_start(wpk_sb[:], wpk.rearrange("(o p) f -> p o f", p=P))
            wqq_sb = wpk_sb.rearrange("p o (w f) -> p o w f", w=3)[:, :, 0]
            wkv_sb = wpk_sb.rearrange("p o (w f) -> p o w f", w=3)[:, :, 1]
            wvk_sb = wpk_sb.rearrange("p o (w f) -> p o w f", w=3)[:, :, 2]
            tmask = consts.tile([P, P], BF16, tag="tmask")
            nc.sync.dma_start(tmask[:], tailmask[:])

            # all x chunk DMAs up front (descending), on the sync queue
            xtiles = [None] * NCHUNK
            for j in reversed(range(NCHUNK)):
                xtiles[j] = xp.tile(
                    [P, CE // P, 512], BF16, tag="xtile", name=f"x{j}"
                )
                nc.sync.dma_start(
                    xtiles[j][:],
                    xT[:, 512 * j:512 * (j + 1)].rearrange("(o p) f -> p o f", p=P),
                )

            # ---- gpsimd-built mask constants ----
            # atri[ch, p] = 1[ch < p];   bneg[ch, c] = M0 * 1[c <= ch]
            #   => (atri^T @ bneg)[p, c] = M0 * max(0, p - c)   (A-block mask)
            # atriB[ch, p] = 1[ch <= p]; bnegB[ch, c] = M0 * 1[c <= ch + 255]
            #   => (atriB^T @ bnegB)[p, c] < 0 iff c < 256 + p  (B-block mask)
            ones = consts.tile([P, P], BF16, tag="ones")
            nc.gpsimd.memset(ones[:], 1.0)
            atri = consts.tile([P, P], BF16, tag="atri")
            nc.gpsimd.memset(atri[:], 1.0)
            nc.gpsimd.affine_select(
                out=atri[:], in_=atri[:],
                compare_op=mybir.AluOpType.is_ge, fill=0.0,
                base=-1, pattern=[[1, P]], channel_multiplier=-1,
            )
            atriB = consts.tile([P, P], BF16, tag="atriB")
            nc.gpsimd.memset(atriB[:], 1.0)
            nc.gpsimd.affine_select(
                out=atriB[:], in_=atriB[:],
                compare_op=mybir.AluOpType.is_ge, fill=0.0,
                base=0, pattern=[[1, P]], channel_multiplier=-1,
            )
            bneg = consts.tile([P, 2 * P], BF16, tag="bneg")
            nc.gpsimd.memset(bneg[:], M0)
            nc.gpsimd.affine_select(
                out=bneg[:], in_=bneg[:],
                compare_op=mybir.AluOpType.is_ge, fill=0.0,
                base=0, pattern=[[-1, 2 * P]], channel_multiplier=1,
            )
            bnegB = consts.tile([P, 3 * P], BF16, tag="bnegB")
            nc.gpsimd.memset(bnegB[:], M0)
            nc.gpsimd.affine_select(
                out=bnegB[:], in_=bnegB[:],
                compare_op=mybir.AluOpType.is_ge, fill=0.0,
                base=255, pattern=[[-1, 3 * P]], channel_multiplier=1,
            )

            # ---- persistent activations ----
            # qTd: q^T duplicated across partition halves (B scores read 64:128)
            qTd = qkv.tile([P, T], BF16, tag="qTd")
            # kv[:, j, 0:128]: [0:64]=kT blk 2j,   [64:128]=vT blk 2j
            # kv[:, j, 128:256]: [0:64]=vT blk 2j+1, [64:128]=kT blk 2j+1
            kv = qkv.tile([P, NCHUNK, 2 * P], BF16, tag="kv")
            vnat = qkv.tile([P, NB, CH], BF16, tag="vnat")
            stats = qkv.tile([P, NCHUNK, 4, 2], F32, tag="stats")
            ssum = qkv.tile([P, NB], F32, tag="ssum")
            rr = qkv.tile([P, NB], F32, tag="rr")
            outsb = qkv.tile([P, 4 * 512], F32, tag="outsb")

            # w2 pair tiles: w2[j][:, 0, :] = block 2j, [:, 1, :] = block 2j+1,
            # both spanning q in [512j, T)
            w2 = [
                w2pool.tile([P, 2, T - 512 * j], BF16, tag=f"w2_{j}", name=f"w2_{j}")
                for j in range(NCHUNK)
            ]

            # PE warm-up spam keeps the HAM clock-gate opening while the
            # first DMAs land.
            # Calibrated warm-up burst: ~10-11us of dependency-free 256-col
            # matmuls keep the PE continuously busy (HAM clock-gate opens
            # after ~3.4us and STAYS open) until the first x chunk lands, so
            # the first projections run at 2.4 GHz instead of 1.2.
            for t in range(40):
                dscr = pop.tile([P, 512], F32, tag="po", name=f"warm{t}")
                nc.tensor.matmul(
                    dscr[0:P, 0:256], qTd[0:P, 0:P], qTd[0:P, 0:256],
                    start=True, stop=True,
                )
            dscr = pop.tile([P, 512], F32, tag="po", name="abs_tm")
            nc.tensor.matmul(
                dscr[0:1, 0:1], tmask[0:CH, 0:1], tmask[0:CH, 0:1],
                start=True, stop=True,
            )

            # preload the exp table set (~2.7us) during the ramp
            nc.scalar.activation(
                stats[0:1, 7, 3, 0:1], ones[0:1, 0:1],
                mybir.ActivationFunctionType.Exp, scale=1.0,
            )

            # filler queue: thunks emitting one output-stripe matmul each,
            # consumed between the last pairs' score tiles to keep PE fed
            fillers = []

            deferred_sums = []
            pending_fin = []

            def drain_fillers(k):
                while k > 0 and fillers:
                    fillers.pop(0)()
                    k -= 1

            def emit_pair(j, fill=0):
                # Blocks A (rows 0:63) and B (rows 64:127) get separate
                # contiguous 1024-wide psum tiles, processed in 2-tile groups.
                # Per-block ACTIVATEs then carry a correct fused accumulator;
                # sums alternate between ACT accum and DVE cache-reduce so
                # neither engine owns all of them.
                L = T - 512 * j
                nt = L // ET
                ngroups = (nt + 1) // 2
                # block B's first 256 cols are always fully masked: provide
                # the zeros via gpsimd and skip them in the exp activate
                nc.gpsimd.memset(w2[j][:, 1, 0:256], 0.0)
                for gi in range(ngroups):
                    t0 = 2 * gi
                    gw = min(2, nt - t0)
                    wA = ET * gw
                    scA = scp.tile([P, 2 * ET], F32, tag="scA",
                                   name=f"scA{j}_{gi}")
                    scB = scp.tile([P, 2 * ET], F32, tag="scB",
                                   name=f"scB{j}_{gi}")
                    pend = []  # (bankkey, kwargs)
                    for u in range(gw):
                        qs = 512 * j + ET * (t0 + u)
                        pend.append((("A", u), dict(
                            out=scA[:, ET * u:ET * (u + 1)],
                            lhsT=kv[0:CH, j, 0:P],
                            rhs=qTd[0:CH, qs:qs + ET], start=True)))
                        pend.append((("B", u), dict(
                            out=scB[:, ET * u:ET * (u + 1)],
                            lhsT=kv[CH:P, j, P:2 * P],
                            rhs=qTd[CH:P, qs:qs + ET], start=True)))
                    if t0 == 0:
                        pend.append((("A", 0), dict(out=scA[:, 0:256],
                                                    lhsT=atri[:],
                                                    rhs=bneg[:], start=False)))
                        pend.append((("B", 0), dict(out=scB[:, 0:3 * P],
                                                    lhsT=atriB[:],
                                                    rhs=bnegB[:],
                                                    start=False)))
                    if t0 + gw == nt:
                        pend.append((("A", gw - 1), dict(
                            out=scA[:, wA - P:wA], lhsT=ones[:],
                            rhs=tmask[:], start=False)))
                        pend.append((("B", gw - 1), dict(
                            out=scB[:, wA - P:wA], lhsT=ones[:],
                            rhs=tmask[:], start=False)))
                    last_per_bank = {}
                    for idx, (bank, kw) in enumerate(pend):
                        last_per_bank[bank] = idx
                    lasts = set(last_per_bank.values())
                    for idx, (bank, kw) in enumerate(pend):
                        nc.tensor.matmul(
                            kw["out"], kw["lhsT"], kw["rhs"],
                            start=kw["start"], stop=(idx in lasts),
                            skip_group_check=True,
                        )
                    lo = ET * t0
                    use_act = ((j + gi) % 2 == 1)
                    for a, sct in ((0, scA), (1, scB)):
                        skip = 256 if (a == 1 and gi == 0) else 0
                        nc.scalar.activation(
                            w2[j][:, a, lo + skip:lo + wA],
                            sct[:, skip:wA],
                            mybir.ActivationFunctionType.Exp, scale=SCALE,
                            accum_out=(stats[:, j, gi, a:a + 1]
                                       if use_act else None),
                        )
                        if not use_act:
                            if gi == ngroups - 1:
                                # deferred with the finalize, so the next
                                # chunk's casts lead the DVE queue
                                deferred_sums.append(
                                    (j, a, lo, wA, gi))
                            else:
                                nc.vector.tensor_scalar(
                                    out=w2[j][:, a, lo:lo + wA],
                                    in0=w2[j][:, a, lo:lo + wA],
                                    scalar1=1.0, scalar2=None,
                                    op0=mybir.AluOpType.mult,
                                    op1=mybir.AluOpType.add,
                                    accum_out=stats[:, j, gi, a:a + 1],
                                )
                    drain_fillers(fill)

                def finalize():
                    for (jj, a, lo, wA, gi) in deferred_sums[:]:
                        if jj != j:
                            continue
                        deferred_sums.remove((jj, a, lo, wA, gi))
                        nc.vector.tensor_scalar(
                            out=w2[j][:, a, lo:lo + wA],
                            in0=w2[j][:, a, lo:lo + wA],
                            scalar1=1.0, scalar2=None,
                            op0=mybir.AluOpType.mult,
                            op1=mybir.AluOpType.add,
                            accum_out=stats[:, j, gi, a:a + 1],
                        )
                    for a in (0, 1):
                        i = 2 * j + a
                        nc.vector.reduce_sum(
                            ssum[:, i:i + 1], stats[:, j, 0:ngroups, a],
                            axis=mybir.AxisListType.X,
                        )
                        nc.vector.reciprocal(rr[:, i:i + 1], ssum[:, i:i + 1])
                        nc.vector.tensor_scalar_mul(
                            vnat[:, i, :], vnat[:, i, :], rr[:, i:i + 1]
                        )
                pending_fin.append(finalize)

            # ======== merged pipeline: chunks descending, scores inline ========
            def process_chunk(j, fill=0):
                xtile = xtiles[j]
                proj = projp.tile([P, 768], F32, tag="proj", name=f"proj{j}")
                # absorber: park this chunk's DMA wait on a throwaway MM
                nc.tensor.matmul(
                    proj[0:1, 0:1], xtile[:, 0, 0:1], xtile[:, 0, 0:1],
                    start=True, stop=True,
                )
                # q projection, duplicated across partition halves: [Wq|Wq]
                for s in range(CE // P):
                    nc.tensor.matmul(
                        proj[:, 0:512], wqq_sb[:, s, :], xtile[:, s, :],
                        start=(s == 0), stop=(s == CE // P - 1),
                        skip_group_check=True,
                    )
                # k/v for the chunk's two key blocks (chunk-local cols 0:128
                # and 256:384): [Wk|Wv] for block A, [Wv|Wk] for block B.
                # The kv matmuls are emitted BEFORE the q cast so the cast's
                # read of proj[0:512] doesn't serialize them (conservative
                # overlap check on the shared proj tile).
                for s in range(CE // P):
                    nc.tensor.matmul(
                        proj[:, 512:640], wkv_sb[:, s, :], xtile[:, s, 0:P],
                        start=(s == 0), stop=(s == CE // P - 1),
                        skip_group_check=True,
                    )
                for s in range(CE // P):
                    nc.tensor.matmul(
                        proj[:, 640:768], wvk_sb[:, s, :], xtile[:, s, 2 * P:3 * P],
                        start=False, stop=(s == CE // P - 1),
                        skip_group_check=True,
                    )
                nc.vector.tensor_copy(qTd[:, 512 * j:512 * (j + 1)], proj[:, 0:512])
                nc.vector.tensor_copy(kv[:, j, :], proj[:, 512:768])
                # previous pair's deferred finalize (sums/reciprocal/v-scale)
                # runs AFTER this chunk's casts so the casts lead the DVE
                # queue and the next scores/exp aren't stalled behind it
                while pending_fin:
                    pending_fin.pop(0)()
                # v -> natural layout via DMA xbar transpose
                teng = nc.sync
                teng.dma_start_transpose(vnat[:, 2 * j, :], kv[CH:P, j, 0:P])
                teng.dma_start_transpose(vnat[:, 2 * j + 1, :], kv[0:CH, j, P:2 * P])
                emit_pair(j, fill=fill)

            # ===== output stripes (definitions; emitted interleaved) =====
            # stripe s covers q [512s, 512s+512); pair pc packs stripe 2pc at
            # psum partitions 0:64 and stripe 2pc+1 at 64:128 of one bank.
            po_tiles = {}
            po_started = {}

            def po_mm(pc, jj, a, s, base, is_last):
                def thunk():
                    if pc not in po_tiles:
                        if pc >= 2:
                            po_tiles[pc] = pop.tile(
                                [P, 512], F32, tag="po", name=f"po{pc}"
                            )
                        else:
                            po_tiles[pc] = scp.tile(
                                [P, 2 * ET], F32,
                                tag=("scA" if pc == 1 else "scB"),
                                name=f"po{pc}",
                            )
                        po_started[pc] = {0: False, 64: False}
                    po = po_tiles[pc]
                    off = 512 * (s - jj)
                    nc.tensor.matmul(
                        po[base:base + CH, 0:512],
                        vnat[:, 2 * jj + a, :],
                        w2[jj][:, a, off:off + 512],
                        start=not po_started[pc][base], stop=is_last,
                        skip_group_check=True,
                    )
                    po_started[pc][base] = True
                return thunk

            def po_flush(pc):
                po = po_tiles[pc]
                nc.vector.tensor_copy(
                    outsb[:, 512 * pc:512 * (pc + 1)], po[:, 0:512]
                )
                nc.sync.dma_start(
                    outT[:, 1024 * pc:1024 * pc + 512],
                    outsb[0:CH, 512 * pc:512 * (pc + 1)],
                )
                nc.sync.dma_start(
                    outT[:, 1024 * pc + 512:1024 * (pc + 1)],
                    outsb[CH:P, 512 * pc:512 * (pc + 1)],
                )

            def po_batch(pc, jj_range):
                out = []
                for jj in jj_range:
                    for a in (0, 1):
                        for s, base in ((2 * pc, 0), (2 * pc + 1, 64)):
                            if jj > s:
                                continue
                            is_last = (jj == 0) and (a == 1) and (
                                True  # jj descending ends at 0
                            )
                            out.append(po_mm(pc, jj, a, s, base, is_last))
                return out

            # pipeline: chunks descending; during the last two pairs feed the
            # PE with po3/po2 contributions from already-finished pairs
            for j in reversed(range(3, NCHUNK)):
                process_chunk(j)
            fillers.extend(po_batch(3, range(7, 2, -1)))  # jj = 7..3
            process_chunk(2, fill=2)
            fillers.extend(po_batch(3, [2]))
            fillers.extend(po_batch(2, range(5, 1, -1)))  # jj = 5..2
            process_chunk(1, fill=3)
            fillers.extend(po_batch(3, [1]))
            fillers.extend(po_batch(2, [1]))
            process_chunk(0, fill=6)
            while pending_fin:
                pending_fin.pop(0)()
            drain_fillers(len(fillers))
            # pair-0 contributions + flushes
            for thunk in po_batch(3, [0]):
                thunk()
            po_flush(3)
            for thunk in po_batch(2, [0]):
                thunk()
            po_flush(2)
            for pc in (1, 0):
                for thunk in po_batch(pc, range(2 * pc + 1, -1, -1)):
                    thunk()
                po_flush(pc)

    return nc


_PROGRAM = None


def _get_program():
    global _PROGRAM
    if _PROGRAM is None:
        nc = _build_program()
        nc.finalize()
        _PROGRAM = nc
    return _PROGRAM


def kernel(x, Wk, Wq, Wv, trace=False, trace_cores=None):
    global LAST_RESULTS
    x = np.asarray(x)
    Wk = np.asarray(Wk)
    Wq = np.asarray(Wq)
    Wv = np.asarray(Wv)

    import ml_dtypes

    bf = ml_dtypes.bfloat16
    wpk = np.ascontiguousarray(
        np.concatenate([Wq, Wq, Wk, Wv, Wv, Wk], axis=1).astype(bf))

    zeros_mask = np.zeros((P, P), bf)
    neg_mask = np.full((P, P), NEG / P, bf)

    in_maps = []
    for c in range(N_CORES):
        b, parity = c // 2, c % 2
        xTb = np.ascontiguousarray(x[b].T).astype(bf)  # [CE, T]
        if parity:
            xTb = np.concatenate([xTb[:, P:], np.zeros((CE, P), bf)], axis=1)
        in_maps.append(
            {
                "xT": np.ascontiguousarray(xTb),
                "wpk": wpk,
                "tailmask": neg_mask if parity else zeros_mask,
            }
        )

    nc = _get_program()
    res = run_bass_kernel_spmd(
        nc,
        in_maps,
        list(range(N_CORES)),
        trace=trace,
        **({"trace_cores": trace_cores} if trace_cores is not None else {}),
    )
    LAST_RESULTS = res

    out = np.zeros((B, T, CH), np.float32)
    for c in range(N_CORES):
        b, parity = c // 2, c % 2
        oT = np.asarray(res.results[c]["outT"], np.float32)  # [CH, T]
        if parity:
            out[b, P:, :] += oT[:, : T - P].T
        else:
            out[b] += oT.T
    return out
